# revision 5
# baseline (speedup 1.0000x reference)
"""RWKV-4 style WKV attention (nn_Attention_4234837754291) on 8 TRN2 NeuronCores.

Self-contained Bass/Tile kernel. Sharding: core i -> (batch b = i//2,
D-half h = i%2). Each core runs the full pipeline for its (b, h): time-mix
projections k/v/r (contract full D, produce its DL=512 output channels), the
linear-space WKV scan over T on those channels, the sigmoid gate, and a
partial output projection through its DL rows of W_out.T. The host sums the
two D-half partial outputs per batch.

Math (linear space; exactly equivalent to the reference's log-space scan):
  y_p[t] = s_p * x[t] + x[t-1]         with s_p = mix_p/(1-mix_p); the (1-mix_p)
                                       factor is folded into the weights
  k = y_k @ Wk_eff, v = y_v @ Wv_eff, r = y_r @ Wr_eff      (bf16 matmuls)
  ek = exp(k)
  A_t = ew*A_{t-1} + ek_t*v_t ;  B_t = ew*B_{t-1} + ek_t    (ew = exp(-exp(td)))
  wkv_t = (A_{t-1} + e^u ek_t v_t)/(B_{t-1} + e^u ek_t)
        = (A_t + c*ekv_t) / (B_t + c*ek_t)                  with c = ew*e^u - 1
  out = (wkv * (1 + tanh(r/2))) @ (0.5 * W_out.T[dsl])      (sigmoid fold)
"""
import os
import numpy as np
import ml_dtypes
from contextlib import ExitStack

import concourse.bacc as bacc
import concourse.tile as tile
import concourse.mybir as mybir
from concourse.bass_utils import run_bass_kernel_spmd

F32 = mybir.dt.float32
BF16 = mybir.dt.bfloat16
AF = mybir.ActivationFunctionType
OP = mybir.AluOpType

B, T, D = 4, 4096, 1024
DL = 512          # D-half per core
TC = 512          # time chunk
NCORES = 8

_NC_CACHE = {}


def _build(D_=D, DL_=DL, T_=T, TC_=TC, n_devices=NCORES):
    KB, MB, NCH = D_ // 128, DL_ // 128, T_ // TC_
    TB = TC_ // 128
    NW = min(512, D_)
    NH = D_ // NW

    nc = bacc.Bacc("TRN2", target_bir_lowering=False, debug=False,
                   num_devices=n_devices)
    x = nc.dram_tensor("x", (T_, D_), BF16, kind="ExternalInput").ap()
    wk = nc.dram_tensor("wk", (D_, DL_), BF16, kind="ExternalInput").ap()
    wv = nc.dram_tensor("wv", (D_, DL_), BF16, kind="ExternalInput").ap()
    wr = nc.dram_tensor("wr", (D_, DL_), BF16, kind="ExternalInput").ap()
    wo = nc.dram_tensor("wo", (DL_, D_), BF16, kind="ExternalInput").ap()
    smix = nc.dram_tensor("smix", (128, 3 * KB), F32, kind="ExternalInput").ap()
    cvec = nc.dram_tensor("cvec", (128, MB), F32, kind="ExternalInput").ap()
    ewb = nc.dram_tensor("ewb", (128, MB), F32, kind="ExternalInput").ap()
    out = nc.dram_tensor("out", (T_, D_), F32, kind="ExternalOutput").ap()

    with tile.TileContext(nc) as tc, ExitStack() as ctx:
        wpool = ctx.enter_context(tc.tile_pool(name="weights", bufs=1))
        wk_sb, wv_sb, wr_sb = [], [], []
        for kb in range(KB):
            for lst, src, nm in ((wk_sb, wk, "wk"), (wv_sb, wv, "wv"),
                                 (wr_sb, wr, "wr")):
                t = wpool.tile([128, DL_], BF16, tag=f"{nm}{kb}")
                nc.sync.dma_start(t[:], src[kb * 128:(kb + 1) * 128, :])
                lst.append(t)
        wo_sb = []
        for mb in range(MB):
            t = wpool.tile([128, D_], BF16, tag=f"wo{mb}")
            nc.sync.dma_start(t[:], wo[mb * 128:(mb + 1) * 128, :])
            wo_sb.append(t)
        smix_sb = wpool.tile([128, 3 * KB], F32, tag="smix")
        nc.sync.dma_start(smix_sb[:], smix[:])
        cvec_sb = wpool.tile([128, MB], F32, tag="cvec")
        nc.sync.dma_start(cvec_sb[:], cvec[:])
        ewb_sb = wpool.tile([128, MB], F32, tag="ewb")
        nc.sync.dma_start(ewb_sb[:], ewb[:])

        xt_pool = ctx.enter_context(tc.tile_pool(name="xt", bufs=2))
        y_pool = ctx.enter_context(tc.tile_pool(name="y", bufs=2))
        pp_pool = ctx.enter_context(tc.tile_pool(name="pp", bufs=4, space="PSUM"))
        po_pool = ctx.enter_context(tc.tile_pool(name="po", bufs=2, space="PSUM"))
        ek_pool = ctx.enter_context(tc.tile_pool(name="ek", bufs=2))
        ekv_pool = ctx.enter_context(tc.tile_pool(name="ekv", bufs=2))
        as_pool = ctx.enter_context(tc.tile_pool(name="as", bufs=2))
        bs_pool = ctx.enter_context(tc.tile_pool(name="bs", bufs=2))
        tr_pool = ctx.enter_context(tc.tile_pool(name="tr", bufs=2))
        ws_pool = ctx.enter_context(tc.tile_pool(name="ws", bufs=2))
        ob_pool = ctx.enter_context(tc.tile_pool(name="ob", bufs=2))
        tmp_pool = ctx.enter_context(tc.tile_pool(name="tmp", bufs=2))

        prevA = [None] * MB
        prevB = [None] * MB
        for c in range(NCH):
            t0 = c * TC_
            xts = []
            OV = 32  # xbar-tile-aligned overlap to supply the t-1 column
            for kb in range(KB):
                xt = xt_pool.tile([128, TC_ + OV], BF16, tag=f"xt{kb}")
                if c == 0:
                    nc.gpsimd.memset(xt[:, OV - 1:OV], 0.0)
                    nc.sync.dma_start_transpose(
                        xt[:, OV:TC_ + OV],
                        x[t0:t0 + TC_, kb * 128:(kb + 1) * 128])
                else:
                    nc.sync.dma_start_transpose(
                        xt[:],
                        x[t0 - OV:t0 + TC_, kb * 128:(kb + 1) * 128])
                xts.append(xt)

            ys = {}
            for pi, p in enumerate(("k", "v", "r")):
                ys[p] = []
                for kb in range(KB):
                    y = y_pool.tile([128, TC_], BF16, tag=f"y{p}{kb}")
                    sc = smix_sb[:, pi * KB + kb: pi * KB + kb + 1]
                    if p == "r":
                        tmp = tmp_pool.tile([128, TC_], BF16, tag=f"tmp{kb}")
                        nc.gpsimd.tensor_scalar(tmp[:], xts[kb][:, 32:TC_ + 32],
                                                sc, None, OP.mult)
                        nc.gpsimd.tensor_tensor(y[:], tmp[:],
                                                xts[kb][:, 31:TC_ + 31], OP.add)
                    else:
                        nc.vector.scalar_tensor_tensor(
                            y[:], xts[kb][:, 32:TC_ + 32], sc,
                            xts[kb][:, 31:TC_ + 31], OP.mult, OP.add)
                    ys[p].append(y)

            wss = []
            for mb in range(MB):
                kp = pp_pool.tile([128, TC_], F32, tag="pp")
                for kb in range(KB):
                    nc.tensor.matmul(kp[:], wk_sb[kb][:, mb * 128:(mb + 1) * 128],
                                     ys["k"][kb][:], start=(kb == 0),
                                     stop=(kb == KB - 1))
                vp = pp_pool.tile([128, TC_], F32, tag="pp")
                for kb in range(KB):
                    nc.tensor.matmul(vp[:], wv_sb[kb][:, mb * 128:(mb + 1) * 128],
                                     ys["v"][kb][:], start=(kb == 0),
                                     stop=(kb == KB - 1))
                rp = pp_pool.tile([128, TC_], F32, tag="pp")
                for kb in range(KB):
                    nc.tensor.matmul(rp[:], wr_sb[kb][:, mb * 128:(mb + 1) * 128],
                                     ys["r"][kb][:], start=(kb == 0),
                                     stop=(kb == KB - 1))

                ek = ek_pool.tile([128, TC_], F32, tag=f"ek{mb}")
                nc.scalar.activation(ek[:], kp[:], AF.Exp)
                tr = tr_pool.tile([128, TC_], BF16, tag=f"tr{mb}")
                nc.scalar.activation(tr[:], rp[:], AF.Tanh, scale=0.5)
                ekv = ekv_pool.tile([128, TC_], F32, tag=f"ekv{mb}")
                nc.vector.tensor_tensor(ekv[:], ek[:], vp[:], OP.mult)

                As = as_pool.tile([128, TC_], F32, tag=f"as{mb}")
                initA = 0.0 if c == 0 else prevA[mb][:, TC_ - 1:TC_]
                ewbc = ewb_sb[:, mb:mb + 1].to_broadcast([128, TC_])
                nc.vector.tensor_tensor_scan(As[:], ewbc, ekv[:], initA,
                                             OP.mult, OP.add)
                Bs = bs_pool.tile([128, TC_], F32, tag=f"bs{mb}")
                initB = 0.0 if c == 0 else prevB[mb][:, TC_ - 1:TC_]
                nc.vector.tensor_tensor_scan(Bs[:], ewbc, ek[:], initB,
                                             OP.mult, OP.add)
                prevA[mb], prevB[mb] = As, Bs

                cs = cvec_sb[:, mb:mb + 1]
                # numer -> ekv ; den -> ek (in-place over dead inputs)
                nc.vector.scalar_tensor_tensor(ekv[:], ekv[:], cs, As[:],
                                               OP.mult, OP.add)
                nc.vector.scalar_tensor_tensor(ek[:], ek[:], cs, Bs[:],
                                               OP.mult, OP.add)
                nc.vector.reciprocal_approx_fast(ek[:], ek[:])
                nc.gpsimd.tensor_tensor(ekv[:], ekv[:], ek[:], OP.mult)  # wkv
                ws = ws_pool.tile([128, TC_], BF16, tag=f"ws{mb}")
                nc.vector.scalar_tensor_tensor(ws[:], tr[:], 1.0, ekv[:],
                                               OP.add, OP.mult)
                wss.append(ws)

            for tb in range(TB):
                po = po_pool.tile([128, D_], F32, tag="po")
                for half in range(NH):
                    for mb in range(MB):
                        nc.tensor.matmul(po[:, half * NW:(half + 1) * NW],
                                         wss[mb][:, tb * 128:(tb + 1) * 128],
                                         wo_sb[mb][:, half * NW:(half + 1) * NW],
                                         start=(mb == 0), stop=(mb == MB - 1))
                ob = ob_pool.tile([128, D_], F32, tag="ob")
                nc.scalar.copy(ob[:], po[:])
                nc.sync.dma_start(out[t0 + tb * 128:t0 + (tb + 1) * 128, :], ob[:])

    nc.compile()
    return nc


def get_nc():
    if "nc" not in _NC_CACHE:
        _NC_CACHE["nc"] = _build()
    return _NC_CACHE["nc"]


def make_in_maps(x, time_decay, time_first, time_mix_k, time_mix_v, time_mix_r,
                 W_key, W_value, W_receptance, W_output):
    x = np.asarray(x, np.float32)
    time_decay = np.asarray(time_decay, np.float64)
    time_first = np.asarray(time_first, np.float64)
    mk = np.asarray(time_mix_k, np.float64).reshape(-1)
    mv = np.asarray(time_mix_v, np.float64).reshape(-1)
    mr = np.asarray(time_mix_r, np.float64).reshape(-1)
    W_key = np.asarray(W_key, np.float32)
    W_value = np.asarray(W_value, np.float32)
    W_receptance = np.asarray(W_receptance, np.float32)
    W_output = np.asarray(W_output, np.float32)

    KB = D // 128
    ew = np.exp(-np.exp(time_decay))
    c = (ew * np.exp(time_first) - 1.0).astype(np.float32)
    ew = ew.astype(np.float32)

    def blocked(vec, nb):
        return np.ascontiguousarray(vec.reshape(nb, 128).T.astype(np.float32))

    smix = np.concatenate([blocked((m / (1.0 - m)), KB) for m in (mk, mv, mr)],
                          axis=1)

    halves = []
    for h in range(2):
        dsl = slice(h * DL, (h + 1) * DL)
        MB = DL // 128

        def eff_w(W, m):
            return np.ascontiguousarray(
                ((1.0 - m)[:, None] * W.T[:, dsl])).astype(ml_dtypes.bfloat16)

        halves.append({
            "wk": eff_w(W_key, mk),
            "wv": eff_w(W_value, mv),
            "wr": eff_w(W_receptance, mr),
            "wo": np.ascontiguousarray(0.5 * W_output.T[dsl, :]).astype(
                ml_dtypes.bfloat16),
            "smix": smix,
            "cvec": blocked(c[dsl], MB),
            "ewb": blocked(ew[dsl], MB),
        })

    in_maps = []
    for i in range(NCORES):
        b, h = i // 2, i % 2
        m = dict(halves[h])
        m["x"] = np.ascontiguousarray(x[b]).astype(ml_dtypes.bfloat16)
        in_maps.append(m)
    return in_maps


def run(in_maps, trace=False):
    nc = get_nc()
    return run_bass_kernel_spmd(nc, in_maps, core_ids=list(range(NCORES)),
                                trace=trace)


def kernel(**inputs):
    in_maps = make_in_maps(**inputs)
    res = run(in_maps, trace=bool(int(os.environ.get("KERNEL_TRACE", "0"))))
    out = np.zeros((B, T, D), np.float32)
    for i in range(NCORES):
        out[i // 2] += res.results[i]["out"]
    if res.exec_time_ns is not None:
        print(f"HW exec time: {res.exec_time_ns} ns")
    return out


# revision 7
# speedup vs baseline: 2.2226x; 2.2226x over previous
"""RWKV-4 style WKV attention (nn_Attention_4234837754291) on 8 TRN2 NeuronCores.

Self-contained Bass/Tile kernel. Sharding: core i -> (batch b = i//2,
D-half h = i%2). Each core runs the full pipeline for its (b, h): time-mix
projections k/v/r (contract full D, produce its DL=512 output channels), the
linear-space WKV scan over T on those channels, the sigmoid gate, and a
partial output projection through its DL rows of W_out.T. The host sums the
two D-half partial outputs per batch.

Math (linear space; exactly equivalent to the reference's log-space scan):
  y_p[t] = s_p * x[t] + x[t-1]         with s_p = mix_p/(1-mix_p); the (1-mix_p)
                                       factor is folded into the weights
  k = y_k @ Wk_eff, v = y_v @ Wv_eff, r = y_r @ Wr_eff      (bf16 matmuls)
  ek = exp(k)
  A_t = ew*A_{t-1} + ek_t*v_t ;  B_t = ew*B_{t-1} + ek_t    (ew = exp(-exp(td)))
  wkv_t = (A_{t-1} + e^u ek_t v_t)/(B_{t-1} + e^u ek_t)
        = (A_t + c*ekv_t) / (B_t + c*ek_t)                  with c = ew*e^u - 1
  out = (wkv * (1 + tanh(r/2))) @ (0.5 * W_out.T[dsl])      (sigmoid fold)
"""
import os
import numpy as np
import ml_dtypes
from contextlib import ExitStack

import concourse.bacc as bacc
import concourse.tile as tile
import concourse.mybir as mybir
from concourse.bass_utils import run_bass_kernel_spmd

F32 = mybir.dt.float32
BF16 = mybir.dt.bfloat16
AF = mybir.ActivationFunctionType
OP = mybir.AluOpType

B, T, D = 4, 4096, 1024
DL = 512          # D-half per core
TC = 512          # time chunk
NCORES = 8

_NC_CACHE = {}


def _build(D_=D, DL_=DL, T_=T, TC_=TC, n_devices=NCORES):
    KB, MB, NCH = D_ // 128, DL_ // 128, T_ // TC_
    TB = TC_ // 128
    NW = min(512, D_)
    NH = D_ // NW

    nc = bacc.Bacc("TRN2", target_bir_lowering=False, debug=False,
                   num_devices=n_devices)
    x = nc.dram_tensor("x", (T_, D_), BF16, kind="ExternalInput").ap()
    wk = nc.dram_tensor("wk", (D_, DL_), BF16, kind="ExternalInput").ap()
    wv = nc.dram_tensor("wv", (D_, DL_), BF16, kind="ExternalInput").ap()
    wr = nc.dram_tensor("wr", (D_, DL_), BF16, kind="ExternalInput").ap()
    wo = nc.dram_tensor("wo", (DL_, D_), BF16, kind="ExternalInput").ap()
    smix = nc.dram_tensor("smix", (128, 3 * KB), F32, kind="ExternalInput").ap()
    cvec = nc.dram_tensor("cvec", (128, MB), F32, kind="ExternalInput").ap()
    ewb = nc.dram_tensor("ewb", (128, MB), F32, kind="ExternalInput").ap()
    out = nc.dram_tensor("out", (T_, D_), F32, kind="ExternalOutput").ap()

    with tile.TileContext(nc) as tc, ExitStack() as ctx:
        wpool = ctx.enter_context(tc.tile_pool(name="weights", bufs=1))
        wk_sb, wv_sb, wr_sb = [], [], []
        for kb in range(KB):
            for lst, src, nm in ((wk_sb, wk, "wk"), (wv_sb, wv, "wv"),
                                 (wr_sb, wr, "wr")):
                t = wpool.tile([128, DL_], BF16, tag=f"{nm}{kb}")
                nc.sync.dma_start(t[:], src[kb * 128:(kb + 1) * 128, :])
                lst.append(t)
        wo_sb = []
        for mb in range(MB):
            t = wpool.tile([128, D_], BF16, tag=f"wo{mb}")
            nc.sync.dma_start(t[:], wo[mb * 128:(mb + 1) * 128, :])
            wo_sb.append(t)
        smix_sb = wpool.tile([128, 3 * KB], F32, tag="smix")
        nc.sync.dma_start(smix_sb[:], smix[:])
        cvec_sb = wpool.tile([128, MB], F32, tag="cvec")
        nc.sync.dma_start(cvec_sb[:], cvec[:])
        ewb_sb = wpool.tile([128, MB], F32, tag="ewb")
        nc.sync.dma_start(ewb_sb[:], ewb[:])

        xt_pool = ctx.enter_context(tc.tile_pool(name="xt", bufs=2))
        y_pool = ctx.enter_context(tc.tile_pool(name="y", bufs=2))
        pp_pool = ctx.enter_context(tc.tile_pool(name="pp", bufs=4, space="PSUM"))
        po_pool = ctx.enter_context(tc.tile_pool(name="po", bufs=2, space="PSUM"))
        ek_pool = ctx.enter_context(tc.tile_pool(name="ek", bufs=2))
        ekv_pool = ctx.enter_context(tc.tile_pool(name="ekv", bufs=2))
        as_pool = ctx.enter_context(tc.tile_pool(name="as", bufs=2))
        bs_pool = ctx.enter_context(tc.tile_pool(name="bs", bufs=2))
        tr_pool = ctx.enter_context(tc.tile_pool(name="tr", bufs=2))
        ws_pool = ctx.enter_context(tc.tile_pool(name="ws", bufs=2))
        ob_pool = ctx.enter_context(tc.tile_pool(name="ob", bufs=2))
        tmp_pool = ctx.enter_context(tc.tile_pool(name="tmp", bufs=1))
        wkv_pool = ctx.enter_context(tc.tile_pool(name="wkv", bufs=1))

        prevA = [None] * MB
        prevB = [None] * MB
        for c in range(NCH):
            t0 = c * TC_
            xts = []
            OV = 32  # xbar-tile-aligned overlap to supply the t-1 column
            for kb in range(KB):
                xt = xt_pool.tile([128, TC_ + OV], BF16, tag=f"xt{kb}")
                if c == 0:
                    nc.gpsimd.memset(xt[:, OV - 1:OV], 0.0)
                    nc.sync.dma_start_transpose(
                        xt[:, OV:TC_ + OV],
                        x[t0:t0 + TC_, kb * 128:(kb + 1) * 128])
                else:
                    nc.sync.dma_start_transpose(
                        xt[:],
                        x[t0 - OV:t0 + TC_, kb * 128:(kb + 1) * 128])
                xts.append(xt)

            ys = {}
            for pi, p in enumerate(("k", "v", "r")):
                ys[p] = []
                for kb in range(KB):
                    y = y_pool.tile([128, TC_], BF16, tag=f"y{p}{kb}")
                    sc = smix_sb[:, pi * KB + kb: pi * KB + kb + 1]
                    tmp = tmp_pool.tile([128, TC_], BF16, tag=f"tmp{kb}")
                    nc.scalar.mul(tmp[:], xts[kb][:, 32:TC_ + 32], sc)
                    nc.vector.tensor_tensor(y[:], tmp[:],
                                            xts[kb][:, 31:TC_ + 31], OP.add)
                    ys[p].append(y)

            wss = []
            for mb in range(MB):
                kp = pp_pool.tile([128, TC_], F32, tag="pp")
                for kb in range(KB):
                    nc.tensor.matmul(kp[:], wk_sb[kb][:, mb * 128:(mb + 1) * 128],
                                     ys["k"][kb][:], start=(kb == 0),
                                     stop=(kb == KB - 1))
                vp = pp_pool.tile([128, TC_], F32, tag="pp")
                for kb in range(KB):
                    nc.tensor.matmul(vp[:], wv_sb[kb][:, mb * 128:(mb + 1) * 128],
                                     ys["v"][kb][:], start=(kb == 0),
                                     stop=(kb == KB - 1))
                rp = pp_pool.tile([128, TC_], F32, tag="pp")
                for kb in range(KB):
                    nc.tensor.matmul(rp[:], wr_sb[kb][:, mb * 128:(mb + 1) * 128],
                                     ys["r"][kb][:], start=(kb == 0),
                                     stop=(kb == KB - 1))

                ek = ek_pool.tile([128, TC_], F32, tag=f"ek{mb}")
                nc.scalar.activation(ek[:], kp[:], AF.Exp)
                tr = tr_pool.tile([128, TC_], BF16, tag=f"tr{mb}")
                nc.scalar.activation(tr[:], rp[:], AF.Tanh, scale=0.5)
                nc.scalar.add(tr[:], tr[:], 1.0)
                ekv = ekv_pool.tile([128, TC_], F32, tag=f"ekv{mb}")
                nc.vector.tensor_tensor(ekv[:], ek[:], vp[:], OP.mult)

                As = as_pool.tile([128, TC_], F32, tag=f"as{mb}")
                initA = 0.0 if c == 0 else prevA[mb][:, TC_ - 1:TC_]
                ewbc = ewb_sb[:, mb:mb + 1].to_broadcast([128, TC_])
                nc.vector.tensor_tensor_scan(As[:], ewbc, ekv[:], initA,
                                             OP.mult, OP.add)
                Bs = bs_pool.tile([128, TC_], F32, tag=f"bs{mb}")
                initB = 0.0 if c == 0 else prevB[mb][:, TC_ - 1:TC_]
                nc.vector.tensor_tensor_scan(Bs[:], ewbc, ek[:], initB,
                                             OP.mult, OP.add)
                prevA[mb], prevB[mb] = As, Bs

                cs = cvec_sb[:, mb:mb + 1]
                # numer -> ekv ; den -> ek (in-place over dead inputs)
                nc.vector.scalar_tensor_tensor(ekv[:], ekv[:], cs, As[:],
                                               OP.mult, OP.add)
                nc.vector.scalar_tensor_tensor(ek[:], ek[:], cs, Bs[:],
                                               OP.mult, OP.add)
                nc.vector.reciprocal_approx_fast(ek[:], ek[:])
                wkv = wkv_pool.tile([128, TC_], BF16, tag=f"wkv{mb}")
                nc.vector.tensor_tensor(wkv[:], ekv[:], ek[:], OP.mult)
                ws = ws_pool.tile([128, TC_], BF16, tag=f"ws{mb}")
                nc.vector.tensor_tensor(ws[:], tr[:], wkv[:], OP.mult)
                wss.append(ws)

            for tb in range(TB):
                po = po_pool.tile([128, D_], F32, tag="po")
                for half in range(NH):
                    for mb in range(MB):
                        nc.tensor.matmul(po[:, half * NW:(half + 1) * NW],
                                         wss[mb][:, tb * 128:(tb + 1) * 128],
                                         wo_sb[mb][:, half * NW:(half + 1) * NW],
                                         start=(mb == 0), stop=(mb == MB - 1))
                ob = ob_pool.tile([128, D_], F32, tag="ob")
                nc.scalar.copy(ob[:], po[:])
                nc.sync.dma_start(out[t0 + tb * 128:t0 + (tb + 1) * 128, :], ob[:])

    nc.compile()
    return nc


def get_nc():
    if "nc" not in _NC_CACHE:
        _NC_CACHE["nc"] = _build()
    return _NC_CACHE["nc"]


def make_in_maps(x, time_decay, time_first, time_mix_k, time_mix_v, time_mix_r,
                 W_key, W_value, W_receptance, W_output):
    x = np.asarray(x, np.float32)
    time_decay = np.asarray(time_decay, np.float64)
    time_first = np.asarray(time_first, np.float64)
    mk = np.asarray(time_mix_k, np.float64).reshape(-1)
    mv = np.asarray(time_mix_v, np.float64).reshape(-1)
    mr = np.asarray(time_mix_r, np.float64).reshape(-1)
    W_key = np.asarray(W_key, np.float32)
    W_value = np.asarray(W_value, np.float32)
    W_receptance = np.asarray(W_receptance, np.float32)
    W_output = np.asarray(W_output, np.float32)

    KB = D // 128
    ew = np.exp(-np.exp(time_decay))
    c = (ew * np.exp(time_first) - 1.0).astype(np.float32)
    ew = ew.astype(np.float32)

    def blocked(vec, nb):
        return np.ascontiguousarray(vec.reshape(nb, 128).T.astype(np.float32))

    smix = np.concatenate([blocked((m / (1.0 - m)), KB) for m in (mk, mv, mr)],
                          axis=1)

    halves = []
    for h in range(2):
        dsl = slice(h * DL, (h + 1) * DL)
        MB = DL // 128

        def eff_w(W, m):
            return np.ascontiguousarray(
                ((1.0 - m)[:, None] * W.T[:, dsl])).astype(ml_dtypes.bfloat16)

        halves.append({
            "wk": eff_w(W_key, mk),
            "wv": eff_w(W_value, mv),
            "wr": eff_w(W_receptance, mr),
            "wo": np.ascontiguousarray(0.5 * W_output.T[dsl, :]).astype(
                ml_dtypes.bfloat16),
            "smix": smix,
            "cvec": blocked(c[dsl], MB),
            "ewb": blocked(ew[dsl], MB),
        })

    in_maps = []
    for i in range(NCORES):
        b, h = i // 2, i % 2
        m = dict(halves[h])
        m["x"] = np.ascontiguousarray(x[b]).astype(ml_dtypes.bfloat16)
        in_maps.append(m)
    return in_maps


def run(in_maps, trace=False):
    nc = get_nc()
    return run_bass_kernel_spmd(nc, in_maps, core_ids=list(range(NCORES)),
                                trace=trace)


def kernel(**inputs):
    in_maps = make_in_maps(**inputs)
    res = run(in_maps, trace=bool(int(os.environ.get("KERNEL_TRACE", "0"))))
    out = np.zeros((B, T, D), np.float32)
    for i in range(NCORES):
        out[i // 2] += res.results[i]["out"]
    if res.exec_time_ns is not None:
        print(f"HW exec time: {res.exec_time_ns} ns")
    return out


# revision 9
# speedup vs baseline: 2.2241x; 1.0007x over previous
"""RWKV-4 style WKV attention (nn_Attention_4234837754291) on 8 TRN2 NeuronCores.

Self-contained Bass/Tile kernel. Sharding: core i -> (batch b = i//2,
D-half h = i%2). Each core runs the full pipeline for its (b, h): time-mix
projections k/v/r (contract full D, produce its DL=512 output channels), the
linear-space WKV scan over T on those channels, the sigmoid gate, and a
partial output projection through its DL rows of W_out.T. The host sums the
two D-half partial outputs per batch.

Math (linear space; exactly equivalent to the reference's log-space scan):
  y_p[t] = s_p * x[t] + x[t-1]         with s_p = mix_p/(1-mix_p); the (1-mix_p)
                                       factor is folded into the weights
  k = y_k @ Wk_eff, v = y_v @ Wv_eff, r = y_r @ Wr_eff      (bf16 matmuls)
  ek = exp(k)
  A_t = ew*A_{t-1} + ek_t*v_t ;  B_t = ew*B_{t-1} + ek_t    (ew = exp(-exp(td)))
  wkv_t = (A_{t-1} + e^u ek_t v_t)/(B_{t-1} + e^u ek_t)
        = (A_t + c*ekv_t) / (B_t + c*ek_t)                  with c = ew*e^u - 1
  out = (wkv * (1 + tanh(r/2))) @ (0.5 * W_out.T[dsl])      (sigmoid fold)
"""
import os
import numpy as np
import ml_dtypes
from contextlib import ExitStack

import concourse.bacc as bacc
import concourse.tile as tile
import concourse.mybir as mybir
from concourse.bass_utils import run_bass_kernel_spmd

F32 = mybir.dt.float32
BF16 = mybir.dt.bfloat16
AF = mybir.ActivationFunctionType
OP = mybir.AluOpType

B, T, D = 4, 4096, 1024
DL = 512          # D-half per core
TC = 512          # time chunk
NCORES = 8

_NC_CACHE = {}


def _build(D_=D, DL_=DL, T_=T, TC_=TC, n_devices=NCORES):
    KB, MB, NCH = D_ // 128, DL_ // 128, T_ // TC_
    TB = TC_ // 128
    NW = min(512, D_)
    NH = D_ // NW

    nc = bacc.Bacc("TRN2", target_bir_lowering=False, debug=False,
                   num_devices=n_devices)
    x = nc.dram_tensor("x", (T_, D_), BF16, kind="ExternalInput").ap()
    wk = nc.dram_tensor("wk", (D_, DL_), BF16, kind="ExternalInput").ap()
    wv = nc.dram_tensor("wv", (D_, DL_), BF16, kind="ExternalInput").ap()
    wr = nc.dram_tensor("wr", (D_, DL_), BF16, kind="ExternalInput").ap()
    wo = nc.dram_tensor("wo", (DL_, D_), BF16, kind="ExternalInput").ap()
    smix = nc.dram_tensor("smix", (128, 3 * KB), F32, kind="ExternalInput").ap()
    cvec = nc.dram_tensor("cvec", (128, MB), F32, kind="ExternalInput").ap()
    ewb = nc.dram_tensor("ewb", (128, MB), F32, kind="ExternalInput").ap()
    out = nc.dram_tensor("out", (T_, D_), F32, kind="ExternalOutput").ap()

    with tile.TileContext(nc) as tc, ExitStack() as ctx:
        wpool = ctx.enter_context(tc.tile_pool(name="weights", bufs=1))
        wk_sb, wv_sb, wr_sb = [], [], []
        for kb in range(KB):
            for lst, src, nm in ((wk_sb, wk, "wk"), (wv_sb, wv, "wv"),
                                 (wr_sb, wr, "wr")):
                t = wpool.tile([128, DL_], BF16, tag=f"{nm}{kb}")
                nc.sync.dma_start(t[:], src[kb * 128:(kb + 1) * 128, :])
                lst.append(t)
        wo_sb = []
        for mb in range(MB):
            t = wpool.tile([128, D_], BF16, tag=f"wo{mb}")
            nc.sync.dma_start(t[:], wo[mb * 128:(mb + 1) * 128, :])
            wo_sb.append(t)
        smix_sb = wpool.tile([128, 3 * KB], F32, tag="smix")
        nc.sync.dma_start(smix_sb[:], smix[:])
        cvec_sb = wpool.tile([128, MB], F32, tag="cvec")
        nc.sync.dma_start(cvec_sb[:], cvec[:])
        ewb_sb = wpool.tile([128, MB], F32, tag="ewb")
        nc.sync.dma_start(ewb_sb[:], ewb[:])

        xt_pool = ctx.enter_context(tc.tile_pool(name="xt", bufs=2))
        y_pool = ctx.enter_context(tc.tile_pool(name="y", bufs=2))
        pp_pool = ctx.enter_context(tc.tile_pool(name="pp", bufs=4, space="PSUM"))
        po_pool = ctx.enter_context(tc.tile_pool(name="po", bufs=1, space="PSUM"))
        ek_pool = ctx.enter_context(tc.tile_pool(name="ek", bufs=2))
        ekv_pool = ctx.enter_context(tc.tile_pool(name="ekv", bufs=2))
        as_pool = ctx.enter_context(tc.tile_pool(name="as", bufs=2))
        bs_pool = ctx.enter_context(tc.tile_pool(name="bs", bufs=2))
        tr_pool = ctx.enter_context(tc.tile_pool(name="tr", bufs=2))
        ws_pool = ctx.enter_context(tc.tile_pool(name="ws", bufs=2))
        ob_pool = ctx.enter_context(tc.tile_pool(name="ob", bufs=2))
        tmp_pool = ctx.enter_context(tc.tile_pool(name="tmp", bufs=1))
        wkv_pool = ctx.enter_context(tc.tile_pool(name="wkv", bufs=1))

        prevA = [None] * MB
        prevB = [None] * MB
        for c in range(NCH):
            t0 = c * TC_
            xts = []
            OV = 32  # xbar-tile-aligned overlap to supply the t-1 column
            for kb in range(KB):
                xt = xt_pool.tile([128, TC_ + OV], BF16, tag=f"xt{kb}")
                if c == 0:
                    nc.gpsimd.memset(xt[:, OV - 1:OV], 0.0)
                    nc.sync.dma_start_transpose(
                        xt[:, OV:TC_ + OV],
                        x[t0:t0 + TC_, kb * 128:(kb + 1) * 128])
                else:
                    nc.sync.dma_start_transpose(
                        xt[:],
                        x[t0 - OV:t0 + TC_, kb * 128:(kb + 1) * 128])
                xts.append(xt)

            ys = {}
            for pi, p in enumerate(("k", "v", "r")):
                ys[p] = []
                for kb in range(KB):
                    y = y_pool.tile([128, TC_], BF16, tag=f"y{p}{kb}")
                    sc = smix_sb[:, pi * KB + kb: pi * KB + kb + 1]
                    tmp = tmp_pool.tile([128, TC_], BF16, tag=f"tmp{kb}")
                    nc.scalar.mul(tmp[:], xts[kb][:, 32:TC_ + 32], sc)
                    nc.vector.tensor_tensor(y[:], tmp[:],
                                            xts[kb][:, 31:TC_ + 31], OP.add)
                    ys[p].append(y)

            wss = []
            for mb in range(MB):
                kp = pp_pool.tile([128, TC_], F32, tag="pp")
                for kb in range(KB):
                    nc.tensor.matmul(kp[:], wk_sb[kb][:, mb * 128:(mb + 1) * 128],
                                     ys["k"][kb][:], start=(kb == 0),
                                     stop=(kb == KB - 1))
                vp = pp_pool.tile([128, TC_], F32, tag="pp")
                for kb in range(KB):
                    nc.tensor.matmul(vp[:], wv_sb[kb][:, mb * 128:(mb + 1) * 128],
                                     ys["v"][kb][:], start=(kb == 0),
                                     stop=(kb == KB - 1))
                rp = pp_pool.tile([128, TC_], F32, tag="pp")
                for kb in range(KB):
                    nc.tensor.matmul(rp[:], wr_sb[kb][:, mb * 128:(mb + 1) * 128],
                                     ys["r"][kb][:], start=(kb == 0),
                                     stop=(kb == KB - 1))

                ek = ek_pool.tile([128, TC_], F32, tag=f"ek{mb}")
                nc.scalar.activation(ek[:], kp[:], AF.Exp)
                tr = tr_pool.tile([128, TC_], BF16, tag=f"tr{mb}")
                nc.scalar.activation(tr[:], rp[:], AF.Tanh, scale=0.5)
                nc.scalar.add(tr[:], tr[:], 1.0)
                ekv = ekv_pool.tile([128, TC_], F32, tag=f"ekv{mb}")
                nc.vector.tensor_tensor(ekv[:], ek[:], vp[:], OP.mult)

                As = as_pool.tile([128, TC_], F32, tag=f"as{mb}")
                initA = 0.0 if c == 0 else prevA[mb][:, TC_ - 1:TC_]
                ewbc = ewb_sb[:, mb:mb + 1].to_broadcast([128, TC_])
                nc.vector.tensor_tensor_scan(As[:], ewbc, ekv[:], initA,
                                             OP.mult, OP.add)
                Bs = bs_pool.tile([128, TC_], F32, tag=f"bs{mb}")
                initB = 0.0 if c == 0 else prevB[mb][:, TC_ - 1:TC_]
                nc.vector.tensor_tensor_scan(Bs[:], ewbc, ek[:], initB,
                                             OP.mult, OP.add)
                prevA[mb], prevB[mb] = As, Bs

                cs = cvec_sb[:, mb:mb + 1]
                # numer -> ekv ; den -> ek (in-place over dead inputs)
                nc.vector.scalar_tensor_tensor(ekv[:], ekv[:], cs, As[:],
                                               OP.mult, OP.add)
                nc.vector.scalar_tensor_tensor(ek[:], ek[:], cs, Bs[:],
                                               OP.mult, OP.add)
                nc.vector.reciprocal_approx_fast(ek[:], ek[:])
                wkv = wkv_pool.tile([128, TC_], BF16, tag=f"wkv{mb}")
                nc.vector.tensor_tensor(wkv[:], ekv[:], ek[:], OP.mult)
                ws = ws_pool.tile([128, TC_], BF16, tag=f"ws{mb}")
                nc.vector.tensor_tensor(ws[:], tr[:], wkv[:], OP.mult)
                wss.append(ws)

            for pair in range(TB // 2):
                pos = [po_pool.tile([128, D_], F32, tag=f"po{i}", name=f"po{i}") for i in range(2)]
                for mb in range(MB):
                    for i, tb in enumerate((pair * 2, pair * 2 + 1)):
                        for half in range(NH):
                            nc.tensor.matmul(
                                pos[i][:, half * NW:(half + 1) * NW],
                                wss[mb][:, tb * 128:(tb + 1) * 128],
                                wo_sb[mb][:, half * NW:(half + 1) * NW],
                                start=(mb == 0), stop=(mb == MB - 1))
                for i, tb in enumerate((pair * 2, pair * 2 + 1)):
                    ob = ob_pool.tile([128, D_], F32, tag="ob")
                    nc.scalar.copy(ob[:], pos[i][:])
                    nc.sync.dma_start(out[t0 + tb * 128:t0 + (tb + 1) * 128, :],
                                      ob[:])

    nc.compile()
    return nc


def get_nc():
    if "nc" not in _NC_CACHE:
        _NC_CACHE["nc"] = _build()
    return _NC_CACHE["nc"]


def make_in_maps(x, time_decay, time_first, time_mix_k, time_mix_v, time_mix_r,
                 W_key, W_value, W_receptance, W_output):
    x = np.asarray(x, np.float32)
    time_decay = np.asarray(time_decay, np.float64)
    time_first = np.asarray(time_first, np.float64)
    mk = np.asarray(time_mix_k, np.float64).reshape(-1)
    mv = np.asarray(time_mix_v, np.float64).reshape(-1)
    mr = np.asarray(time_mix_r, np.float64).reshape(-1)
    W_key = np.asarray(W_key, np.float32)
    W_value = np.asarray(W_value, np.float32)
    W_receptance = np.asarray(W_receptance, np.float32)
    W_output = np.asarray(W_output, np.float32)

    KB = D // 128
    ew = np.exp(-np.exp(time_decay))
    c = (ew * np.exp(time_first) - 1.0).astype(np.float32)
    ew = ew.astype(np.float32)

    def blocked(vec, nb):
        return np.ascontiguousarray(vec.reshape(nb, 128).T.astype(np.float32))

    smix = np.concatenate([blocked((m / (1.0 - m)), KB) for m in (mk, mv, mr)],
                          axis=1)

    halves = []
    for h in range(2):
        dsl = slice(h * DL, (h + 1) * DL)
        MB = DL // 128

        def eff_w(W, m):
            return np.ascontiguousarray(
                ((1.0 - m)[:, None] * W.T[:, dsl])).astype(ml_dtypes.bfloat16)

        halves.append({
            "wk": eff_w(W_key, mk),
            "wv": eff_w(W_value, mv),
            "wr": eff_w(W_receptance, mr),
            "wo": np.ascontiguousarray(0.5 * W_output.T[dsl, :]).astype(
                ml_dtypes.bfloat16),
            "smix": smix,
            "cvec": blocked(c[dsl], MB),
            "ewb": blocked(ew[dsl], MB),
        })

    in_maps = []
    for i in range(NCORES):
        b, h = i // 2, i % 2
        m = dict(halves[h])
        m["x"] = np.ascontiguousarray(x[b]).astype(ml_dtypes.bfloat16)
        in_maps.append(m)
    return in_maps


def run(in_maps, trace=False):
    nc = get_nc()
    return run_bass_kernel_spmd(nc, in_maps, core_ids=list(range(NCORES)),
                                trace=trace)


def kernel(**inputs):
    in_maps = make_in_maps(**inputs)
    res = run(in_maps, trace=bool(int(os.environ.get("KERNEL_TRACE", "0"))))
    out = np.zeros((B, T, D), np.float32)
    for i in range(NCORES):
        out[i // 2] += res.results[i]["out"]
    if res.exec_time_ns is not None:
        print(f"HW exec time: {res.exec_time_ns} ns")
    return out


# revision 10
# speedup vs baseline: 2.4714x; 1.1112x over previous
"""RWKV-4 style WKV attention (nn_Attention_4234837754291) on 8 TRN2 NeuronCores.

Self-contained Bass/Tile kernel. Sharding: core i -> (batch b = i//2,
D-half h = i%2). Each core runs the full pipeline for its (b, h): time-mix
projections k/v/r (contract full D, produce its DL=512 output channels), the
linear-space WKV scan over T on those channels, the sigmoid gate, and a
partial output projection through its DL rows of W_out.T. The host sums the
two D-half partial outputs per batch.

Math (linear space; exactly equivalent to the reference's log-space scan):
  y_p[t] = s_p * x[t] + x[t-1]         with s_p = mix_p/(1-mix_p); the (1-mix_p)
                                       factor is folded into the weights
  k = y_k @ Wk_eff, v = y_v @ Wv_eff, r = y_r @ Wr_eff      (bf16 matmuls)
  ek = exp(k)
  A_t = ew*A_{t-1} + ek_t*v_t ;  B_t = ew*B_{t-1} + ek_t    (ew = exp(-exp(td)))
  wkv_t = (A_{t-1} + e^u ek_t v_t)/(B_{t-1} + e^u ek_t)
        = (A_t + c*ekv_t) / (B_t + c*ek_t)                  with c = ew*e^u - 1
  out = (wkv * (1 + tanh(r/2))) @ (0.5 * W_out.T[dsl])      (sigmoid fold)
"""
import os
import numpy as np
import ml_dtypes
from contextlib import ExitStack

import concourse.bacc as bacc
import concourse.tile as tile
import concourse.mybir as mybir
from concourse.bass_utils import run_bass_kernel_spmd

F32 = mybir.dt.float32
BF16 = mybir.dt.bfloat16
AF = mybir.ActivationFunctionType
OP = mybir.AluOpType

B, T, D = 4, 4096, 1024
DL = 512          # D-half per core
TC = 512          # time chunk
NCORES = 8

_NC_CACHE = {}


def _build(D_=D, DL_=DL, T_=T, TC_=TC, n_devices=NCORES):
    KB, MB, NCH = D_ // 128, DL_ // 128, T_ // TC_
    TB = TC_ // 128
    NW = min(512, D_)
    NH = D_ // NW

    nc = bacc.Bacc("TRN2", target_bir_lowering=False, debug=False,
                   num_devices=n_devices)
    x = nc.dram_tensor("x", (T_, D_), BF16, kind="ExternalInput").ap()
    wk = nc.dram_tensor("wk", (D_, DL_), BF16, kind="ExternalInput").ap()
    wv = nc.dram_tensor("wv", (D_, DL_), BF16, kind="ExternalInput").ap()
    wr = nc.dram_tensor("wr", (D_, DL_), BF16, kind="ExternalInput").ap()
    wo = nc.dram_tensor("wo", (DL_, D_), BF16, kind="ExternalInput").ap()
    smix = nc.dram_tensor("smix", (128, 3 * KB), F32, kind="ExternalInput").ap()
    cvec = nc.dram_tensor("cvec", (128, MB), F32, kind="ExternalInput").ap()
    ewb = nc.dram_tensor("ewb", (128, MB), F32, kind="ExternalInput").ap()
    out = nc.dram_tensor("out", (T_, D_), F32, kind="ExternalOutput").ap()

    with tile.TileContext(nc) as tc, ExitStack() as ctx:
        wpool = ctx.enter_context(tc.tile_pool(name="weights", bufs=1))
        wk_sb, wv_sb, wr_sb = [], [], []
        for kb in range(KB):
            for lst, src, nm in ((wk_sb, wk, "wk"), (wv_sb, wv, "wv"),
                                 (wr_sb, wr, "wr")):
                t = wpool.tile([128, DL_], BF16, tag=f"{nm}{kb}")
                nc.sync.dma_start(t[:], src[kb * 128:(kb + 1) * 128, :])
                lst.append(t)
        wo_sb = []
        for mb in range(MB):
            t = wpool.tile([128, D_], BF16, tag=f"wo{mb}")
            nc.sync.dma_start(t[:], wo[mb * 128:(mb + 1) * 128, :])
            wo_sb.append(t)
        smix_sb = wpool.tile([128, 3 * KB], F32, tag="smix")
        nc.sync.dma_start(smix_sb[:], smix[:])
        cvec_sb = wpool.tile([128, MB], F32, tag="cvec")
        nc.sync.dma_start(cvec_sb[:], cvec[:])
        ewb_sb = wpool.tile([128, MB], F32, tag="ewb")
        nc.sync.dma_start(ewb_sb[:], ewb[:])

        xt_pool = ctx.enter_context(tc.tile_pool(name="xt", bufs=2))
        y_pool = ctx.enter_context(tc.tile_pool(name="y", bufs=2))
        pp_pool = ctx.enter_context(tc.tile_pool(name="pp", bufs=4, space="PSUM"))
        po_pool = ctx.enter_context(tc.tile_pool(name="po", bufs=1, space="PSUM"))
        ek_pool = ctx.enter_context(tc.tile_pool(name="ek", bufs=2))
        ekv_pool = ctx.enter_context(tc.tile_pool(name="ekv", bufs=2))
        as_pool = ctx.enter_context(tc.tile_pool(name="as", bufs=2))
        bs_pool = ctx.enter_context(tc.tile_pool(name="bs", bufs=2))
        tr_pool = ctx.enter_context(tc.tile_pool(name="tr", bufs=2))
        ws_pool = ctx.enter_context(tc.tile_pool(name="ws", bufs=2))
        ob_pool = ctx.enter_context(tc.tile_pool(name="ob", bufs=2))
        tmp_pool = ctx.enter_context(tc.tile_pool(name="tmp", bufs=1))
        wkv_pool = ctx.enter_context(tc.tile_pool(name="wkv", bufs=1))

        def hot(inst, boost=600):
            if inst is not None and inst.ins.bass_priority is not None:
                inst.ins.bass_priority -= boost
            return inst

        prevA = [None] * MB
        prevB = [None] * MB
        for c in range(NCH):
            t0 = c * TC_
            xts = []
            OV = 32  # xbar-tile-aligned overlap to supply the t-1 column
            for kb in range(KB):
                xt = xt_pool.tile([128, TC_ + OV], BF16, tag=f"xt{kb}")
                if c == 0:
                    nc.gpsimd.memset(xt[:, OV - 1:OV], 0.0)
                    nc.sync.dma_start_transpose(
                        xt[:, OV:TC_ + OV],
                        x[t0:t0 + TC_, kb * 128:(kb + 1) * 128])
                else:
                    nc.sync.dma_start_transpose(
                        xt[:],
                        x[t0 - OV:t0 + TC_, kb * 128:(kb + 1) * 128])
                xts.append(xt)

            ys = {}
            for pi, p in enumerate(("k", "v", "r")):
                ys[p] = []
                for kb in range(KB):
                    y = y_pool.tile([128, TC_], BF16, tag=f"y{p}{kb}")
                    sc = smix_sb[:, pi * KB + kb: pi * KB + kb + 1]
                    tmp = tmp_pool.tile([128, TC_], BF16, tag=f"tmp{kb}")
                    nc.scalar.mul(tmp[:], xts[kb][:, 32:TC_ + 32], sc)
                    nc.vector.tensor_tensor(y[:], tmp[:],
                                            xts[kb][:, 31:TC_ + 31], OP.add)
                    ys[p].append(y)

            wss = []
            for mb in range(MB):
                kp = pp_pool.tile([128, TC_], F32, tag="pp")
                for kb in range(KB):
                    nc.tensor.matmul(kp[:], wk_sb[kb][:, mb * 128:(mb + 1) * 128],
                                     ys["k"][kb][:], start=(kb == 0),
                                     stop=(kb == KB - 1))
                vp = pp_pool.tile([128, TC_], F32, tag="pp")
                for kb in range(KB):
                    nc.tensor.matmul(vp[:], wv_sb[kb][:, mb * 128:(mb + 1) * 128],
                                     ys["v"][kb][:], start=(kb == 0),
                                     stop=(kb == KB - 1))
                rp = pp_pool.tile([128, TC_], F32, tag="pp")
                for kb in range(KB):
                    nc.tensor.matmul(rp[:], wr_sb[kb][:, mb * 128:(mb + 1) * 128],
                                     ys["r"][kb][:], start=(kb == 0),
                                     stop=(kb == KB - 1))

                ek = ek_pool.tile([128, TC_], F32, tag=f"ek{mb}")
                hot(nc.scalar.activation(ek[:], kp[:], AF.Exp))
                tr = tr_pool.tile([128, TC_], BF16, tag=f"tr{mb}")
                hot(nc.scalar.activation(tr[:], rp[:], AF.Tanh, scale=0.5))
                hot(nc.scalar.add(tr[:], tr[:], 1.0))
                ekv = ekv_pool.tile([128, TC_], F32, tag=f"ekv{mb}")
                hot(nc.vector.tensor_tensor(ekv[:], ek[:], vp[:], OP.mult))

                As = as_pool.tile([128, TC_], F32, tag=f"as{mb}")
                initA = 0.0 if c == 0 else prevA[mb][:, TC_ - 1:TC_]
                ewbc = ewb_sb[:, mb:mb + 1].to_broadcast([128, TC_])
                hot(nc.vector.tensor_tensor_scan(As[:], ewbc, ekv[:], initA,
                                             OP.mult, OP.add))
                Bs = bs_pool.tile([128, TC_], F32, tag=f"bs{mb}")
                initB = 0.0 if c == 0 else prevB[mb][:, TC_ - 1:TC_]
                hot(nc.vector.tensor_tensor_scan(Bs[:], ewbc, ek[:], initB,
                                             OP.mult, OP.add))
                prevA[mb], prevB[mb] = As, Bs

                cs = cvec_sb[:, mb:mb + 1]
                # numer -> ekv ; den -> ek (in-place over dead inputs)
                hot(nc.vector.scalar_tensor_tensor(ekv[:], ekv[:], cs, As[:],
                                               OP.mult, OP.add))
                hot(nc.vector.scalar_tensor_tensor(ek[:], ek[:], cs, Bs[:],
                                               OP.mult, OP.add))
                hot(nc.vector.reciprocal_approx_fast(ek[:], ek[:]))
                wkv = wkv_pool.tile([128, TC_], BF16, tag=f"wkv{mb}")
                hot(nc.vector.tensor_tensor(wkv[:], ekv[:], ek[:], OP.mult))
                ws = ws_pool.tile([128, TC_], BF16, tag=f"ws{mb}")
                hot(nc.vector.tensor_tensor(ws[:], tr[:], wkv[:], OP.mult))
                wss.append(ws)

            for pair in range(TB // 2):
                pos = [po_pool.tile([128, D_], F32, tag=f"po{i}", name=f"po{i}") for i in range(2)]
                for mb in range(MB):
                    for i, tb in enumerate((pair * 2, pair * 2 + 1)):
                        for half in range(NH):
                            nc.tensor.matmul(
                                pos[i][:, half * NW:(half + 1) * NW],
                                wss[mb][:, tb * 128:(tb + 1) * 128],
                                wo_sb[mb][:, half * NW:(half + 1) * NW],
                                start=(mb == 0), stop=(mb == MB - 1))
                for i, tb in enumerate((pair * 2, pair * 2 + 1)):
                    ob = ob_pool.tile([128, D_], F32, tag="ob")
                    nc.scalar.copy(ob[:], pos[i][:])
                    nc.sync.dma_start(out[t0 + tb * 128:t0 + (tb + 1) * 128, :],
                                      ob[:])

    nc.compile()
    return nc


def get_nc():
    if "nc" not in _NC_CACHE:
        _NC_CACHE["nc"] = _build()
    return _NC_CACHE["nc"]


def make_in_maps(x, time_decay, time_first, time_mix_k, time_mix_v, time_mix_r,
                 W_key, W_value, W_receptance, W_output):
    x = np.asarray(x, np.float32)
    time_decay = np.asarray(time_decay, np.float64)
    time_first = np.asarray(time_first, np.float64)
    mk = np.asarray(time_mix_k, np.float64).reshape(-1)
    mv = np.asarray(time_mix_v, np.float64).reshape(-1)
    mr = np.asarray(time_mix_r, np.float64).reshape(-1)
    W_key = np.asarray(W_key, np.float32)
    W_value = np.asarray(W_value, np.float32)
    W_receptance = np.asarray(W_receptance, np.float32)
    W_output = np.asarray(W_output, np.float32)

    KB = D // 128
    ew = np.exp(-np.exp(time_decay))
    c = (ew * np.exp(time_first) - 1.0).astype(np.float32)
    ew = ew.astype(np.float32)

    def blocked(vec, nb):
        return np.ascontiguousarray(vec.reshape(nb, 128).T.astype(np.float32))

    smix = np.concatenate([blocked((m / (1.0 - m)), KB) for m in (mk, mv, mr)],
                          axis=1)

    halves = []
    for h in range(2):
        dsl = slice(h * DL, (h + 1) * DL)
        MB = DL // 128

        def eff_w(W, m):
            return np.ascontiguousarray(
                ((1.0 - m)[:, None] * W.T[:, dsl])).astype(ml_dtypes.bfloat16)

        halves.append({
            "wk": eff_w(W_key, mk),
            "wv": eff_w(W_value, mv),
            "wr": eff_w(W_receptance, mr),
            "wo": np.ascontiguousarray(0.5 * W_output.T[dsl, :]).astype(
                ml_dtypes.bfloat16),
            "smix": smix,
            "cvec": blocked(c[dsl], MB),
            "ewb": blocked(ew[dsl], MB),
        })

    in_maps = []
    for i in range(NCORES):
        b, h = i // 2, i % 2
        m = dict(halves[h])
        m["x"] = np.ascontiguousarray(x[b]).astype(ml_dtypes.bfloat16)
        in_maps.append(m)
    return in_maps


def run(in_maps, trace=False):
    nc = get_nc()
    return run_bass_kernel_spmd(nc, in_maps, core_ids=list(range(NCORES)),
                                trace=trace)


def kernel(**inputs):
    in_maps = make_in_maps(**inputs)
    res = run(in_maps, trace=bool(int(os.environ.get("KERNEL_TRACE", "0"))))
    out = np.zeros((B, T, D), np.float32)
    for i in range(NCORES):
        out[i // 2] += res.results[i]["out"]
    if res.exec_time_ns is not None:
        print(f"HW exec time: {res.exec_time_ns} ns")
    return out
